# revision 1
# baseline (speedup 1.0000x reference)
"""Longformer-style 2-layer encoder (S=4096, HID=768, sliding window W=256)
on 8 Trainium2 NeuronCores.

Sharding: sequence-parallel. Core c owns tokens [512c, 512c+512) and holds a
1024-token slab (own 512 + 256-token halo each side) of TRANSPOSED
activations x^T [768 feat, 1024 tok] in SBUF. Layer-1 halos are computed
locally from embeddings; layer-2 halos arrive via an AllGather of layer-1
outputs + per-core indirect gathers (gather indices are per-core input data,
keeping the single SPMD program uniform across cores).

Matmuls run in float32r (full-rate PE mode, ~tf32 precision), fp32 PSUM.
Activations stay transposed so QKV/FFN weights serve as lhsT in natural
DRAM layout. Attention: S^T = (K^T-slice).T @ Q^T per (chunk, head), additive
band/validity mask then exp on ACT; softmax denominators ride the PV matmul
as a 65th all-ones column of V; normalization = K=1 broadcast matmul + DVE
multiply. LayerNorm over the feature (partition) axis uses ones-column
matmuls for stats and K=1 broadcast matmuls for the column-affine apply.
"""
import os
import sys
import types
from contextlib import ExitStack

import ml_dtypes
import numpy as np

# --- optional NTFF profiling shim (antenv.axon_hooks missing in image) ----
try:
    import antenv
    if 'antenv.axon_hooks' not in sys.modules:
        _mod = types.ModuleType('antenv.axon_hooks')
        _hook = [None]
        _mod.set_axon_ntff_profile_hook = lambda h: _hook.__setitem__(0, h)
        _mod.get_axon_ntff_profile_hook = lambda: _hook[0]
        sys.modules['antenv.axon_hooks'] = _mod
        antenv.axon_hooks = _mod
        try:
            from trn_agent_boot.trn_boot import _ntff_profile_via_ctypes
            _mod.set_axon_ntff_profile_hook(
                _ntff_profile_via_ctypes('/opt/axon/libaxon_pjrt.so'))
        except Exception:
            pass
except Exception:
    pass

import concourse.bass as bass
import concourse.mybir as mybir
import concourse.tile as tile
from concourse import bacc
from concourse.bass_utils import run_bass_kernel_spmd
from concourse.masks import make_identity

f32 = mybir.dt.float32
f32r = mybir.dt.float32r
bf16 = mybir.dt.bfloat16
i32 = mybir.dt.int32
AF = mybir.ActivationFunctionType
ALU = mybir.AluOpType

NCORES = 8
P = 128
S, HID, NH, HD, FF, L = 4096, 768, 12, 64, 3072, 2
W = 256
SL = 512
SLAB = 1024
FB = HID // P     # 6
TB = SLAB // P    # 8
FFB = FF // P     # 24
EPS = 1e-5
NEG = -1e9

_cache = {}


def _ln_T(nc, sbp, psp, ones, src, dst_ap, g_ap, b_ap):
    """LayerNorm over the 768-feature partition axis of transposed
    activations src [128, FB, 512] (f32r). dst_ap(b) -> out AP block."""
    srow = psp.tile([1, SL], f32, tag="ps", name="srow")
    qrow = psp.tile([1, SL], f32, tag="ps", name="qrow")
    for b in range(FB):
        sq = sbp.tile([P, SL], f32r, tag="lnsq", name=f"lnsq{b}")
        nc.scalar.activation(sq[:], src[:, b, :].bitcast(f32), AF.Square)
        nc.tensor.matmul(srow[:], ones[:, 0:1], src[:, b, :],
                         start=(b == 0), stop=(b == FB - 1))
        nc.tensor.matmul(qrow[:], ones[:, 0:1], sq[:],
                         start=(b == 0), stop=(b == FB - 1))
    mean = sbp.tile([1, SL], f32, tag="lnrow_m", bufs=1, name="mean")
    var = sbp.tile([1, SL], f32, tag="lnrow_v", bufs=1, name="var")
    c1 = sbp.tile([1, SL], f32r, tag="lnrow_c1", bufs=1, name="c1")
    c0 = sbp.tile([1, SL], f32r, tag="lnrow_c0", bufs=1, name="c0")
    tmpm = sbp.tile([1, SL], f32, tag="lnrow_t", bufs=1, name="tmpm")
    nc.vector.tensor_scalar_mul(mean[:], srow[:], 1.0 / HID)
    nc.vector.tensor_tensor(var[:], mean[:], mean[:], op=ALU.mult)
    nc.vector.tensor_scalar(var[:], var[:], -1.0, EPS, op0=ALU.mult, op1=ALU.add)
    qtmp = sbp.tile([1, SL], f32, tag="lnrow_q", bufs=1, name="qtmp")
    nc.vector.tensor_scalar_mul(qtmp[:], qrow[:], 1.0 / HID)
    nc.vector.tensor_tensor(var[:], qtmp[:], var[:], op=ALU.add)
    nc.vector.reciprocal(var[:], var[:])
    nc.scalar.activation(c1[:], var[:], AF.Sqrt)          # rstd, f32r
    nc.vector.tensor_tensor(tmpm[:], mean[:], c1[:].bitcast(f32), op=ALU.mult)
    nc.scalar.activation(c0[:], tmpm[:], AF.Identity, bias=0.0, scale=-1.0)
    c1b = psp.tile([P, SL], f32, tag="ps", name="c1b")
    c0b = psp.tile([P, SL], f32, tag="ps", name="c0b")
    nc.tensor.matmul(c1b[:], ones[0:1, :], c1[:], start=True, stop=True)
    nc.tensor.matmul(c0b[:], ones[0:1, :], c0[:], start=True, stop=True)
    for b in range(FB):
        t = sbp.tile([P, SL], f32, tag="lnap", name=f"lnap{b}")
        nc.vector.tensor_tensor(t[:], src[:, b, :].bitcast(f32), c1b[:],
                                op=ALU.mult)
        nc.vector.tensor_tensor(t[:], t[:], c0b[:], op=ALU.add)
        nc.scalar.activation(dst_ap(b), t[:], AF.Identity,
                             bias=b_ap(b), scale=g_ap(b))


def build(stage=5):
    if stage in _cache:
        return _cache[stage]
    nc = bacc.Bacc("TRN2", target_bir_lowering=False, debug=False,
                   num_devices=NCORES)

    xemb_d = nc.dram_tensor("xemb", [SLAB, HID], f32, kind="ExternalInput")
    lnemb_d = nc.dram_tensor("lnemb", [P, 2, FB], f32, kind="ExternalInput")
    madd_d = nc.dram_tensor("madd", [P, 2, FB, 256], bf16, kind="ExternalInput")
    hidx_d = nc.dram_tensor("hidx", [P, 12], i32, kind="ExternalInput")
    ones_d = nc.dram_tensor("onesr", [P, P], f32r, kind="ExternalInput")
    wd, ppd, b1d = [], [], []
    for l in range(L):
        wd.append({k: nc.dram_tensor(f"{k}{l}", shp, f32r, kind="ExternalInput")
                   for k, shp in [("wq", [HID, HID]), ("wk", [HID, HID]),
                                  ("wv", [HID, HID]), ("wo", [HID, HID]),
                                  ("w1", [HID, FF]), ("w2", [FF, HID])]})
        ppd.append(nc.dram_tensor(f"pp{l}", [P, 8, FB], f32, kind="ExternalInput"))
        b1d.append(nc.dram_tensor(f"b1_{l}", [P, FFB], f32, kind="ExternalInput"))

    xout_d = nc.dram_tensor("xout", [FB, P, SL], f32, kind="ExternalOutput")
    dbg_d = (nc.dram_tensor("dbg", [P, FB, SLAB], f32, kind="ExternalOutput")
             if stage != 5 else None)

    with tile.TileContext(nc) as tc, ExitStack() as top:
        constp = top.enter_context(tc.tile_pool(name="const", bufs=1))
        sbp = top.enter_context(tc.tile_pool(name="sb", bufs=2))
        bigp = top.enter_context(tc.tile_pool(name="big", bufs=1))
        wp = top.enter_context(tc.tile_pool(name="wp", bufs=2))
        psp = top.enter_context(tc.tile_pool(name="ps", bufs=7, space="PSUM"))
        dramp = top.enter_context(tc.tile_pool(name="dram", bufs=1, space="DRAM"))

        ident = constp.tile([P, P], f32)
        make_identity(nc, ident[:])
        ones = constp.tile([P, P], f32r)
        nc.sync.dma_start(ones[:], ones_d[:])
        lnemb = constp.tile([P, 2, FB], f32)
        nc.sync.dma_start(lnemb[:], lnemb_d[:])
        madd = constp.tile([P, 2, FB, 256], bf16)
        nc.sync.dma_start(madd[:], madd_d[:])
        hidx = constp.tile([P, 12], i32)
        nc.sync.dma_start(hidx[:], hidx_d[:])
        pp, b1t = [], []
        for l in range(L):
            ppt = constp.tile([P, 8, FB], f32, name=f"pp{l}")
            nc.sync.dma_start(ppt[:], ppd[l][:])
            pp.append(ppt)
            b1 = constp.tile([P, FFB], f32, name=f"b1_{l}")
            nc.sync.dma_start(b1[:], b1d[l][:])
            b1t.append(b1)

        def wload(dram_ap, name):
            t = wp.tile([P, FB, 384], f32r, tag="wt", name=name)
            nc.sync.dma_start(t[:], dram_ap.rearrange("(kb kp) m -> kp kb m", kp=P))
            return t

        # ================= embedding: natural LN + transpose ==============
        slab = bigp.tile([P, FB, SLAB], f32r, tag="slab", bufs=2, name="slab1")
        if stage == 0:
            # minimal: xemb -> slab (reinterpret) -> dbg
            for tb in range(TB):
                xe0 = sbp.tile([P, HID], f32, tag="xe0", name=f"xe0_{tb}")
                nc.sync.dma_start(xe0[:], xemb_d[tb * P:(tb + 1) * P, :])
                nc.vector.tensor_copy(slab[:, 0, tb * P:(tb + 1) * P].bitcast(f32),
                                      xe0[:, 0:P])
            nc.sync.dma_start(dbg_d[:], slab[:].bitcast(f32))
        with tc.tile_pool(name="embp", bufs=2) as ep:
            for tb in range(TB if stage != 0 else 0):
                xe = ep.tile([P, HID], f32, tag="xe", name=f"xe{tb}")
                nc.sync.dma_start(xe[:], xemb_d[tb * P:(tb + 1) * P, :])
                nm = ep.tile([P, 1], f32, tag="enm", name=f"nm{tb}")
                nc.vector.reduce_sum(out=nm[:], in_=xe[:],
                                     axis=mybir.AxisListType.X)
                nc.vector.tensor_scalar_mul(nm[:], nm[:], -1.0 / HID)
                xc = ep.tile([P, HID], f32, tag="exc", name=f"xc{tb}")
                nc.scalar.activation(xc[:], xe[:], AF.Identity, bias=nm[:, 0:1])
                sqs = ep.tile([P, HID], f32, tag="esq", name=f"sqs{tb}")
                var = ep.tile([P, 1], f32, tag="evar", name=f"var{tb}")
                rstd = ep.tile([P, 1], f32, tag="ers", name=f"rstd{tb}")
                nc.scalar.activation(sqs[:], xc[:], AF.Square)
                nc.vector.reduce_sum(out=var[:], in_=sqs[:],
                                     axis=mybir.AxisListType.X)
                nc.vector.tensor_scalar(var[:], var[:], 1.0 / HID, EPS,
                                        op0=ALU.mult, op1=ALU.add)
                nc.vector.reciprocal(var[:], var[:])
                nc.scalar.activation(rstd[:], var[:], AF.Sqrt)
                xn = ep.tile([P, HID], f32, tag="exn", name=f"xn{tb}")
                nc.scalar.activation(xn[:], xc[:], AF.Identity,
                                     scale=rstd[:, 0:1])
                for b in range(FB):
                    tp = psp.tile([P, P], f32, tag="ps", name=f"tp{tb}_{b}")
                    nc.tensor.transpose(tp[:], xn[:, b * P:(b + 1) * P], ident[:])
                    nc.scalar.activation(
                        slab[:, b, tb * P:(tb + 1) * P], tp[:], AF.Identity,
                        bias=lnemb[:, 1, b:b + 1], scale=lnemb[:, 0, b:b + 1])

        if stage == 1:
            nc.sync.dma_start(dbg_d[:], slab[:].bitcast(f32))

        # ======================= transformer layers =======================
        for l in range(L):
            if stage <= 1:
                break
            xown = slab[:, :, W:W + SL]

            res1 = bigp.tile([P, FB, SL], f32r, tag="res", name=f"res1_{l}")
            with tc.tile_pool(name=f"attn{l}", bufs=1) as ap:
                # ---- v (natural, ones-augmented) ----
                vt = ap.tile([P, TB, NH, HD + 1], f32r, tag="vt", name=f"vt{l}")
                nc.sync.dma_start(
                    vt[:, :, :, HD:HD + 1],
                    ones_d[:, 0:TB * NH].rearrange("p (a b c) -> p a b c",
                                                   a=TB, b=NH))
                for nn in range(2):
                    wvh = wload(wd[l]["wv"][:, nn * 384:(nn + 1) * 384],
                                f"wv{l}_{nn}")
                    for tb in range(TB):
                        pv = psp.tile([P, 384], f32, tag="ps",
                                      name=f"pvv{l}_{nn}_{tb}")
                        for kb in range(FB):
                            nc.tensor.matmul(
                                pv[:], slab[:, kb, tb * P:(tb + 1) * P],
                                wvh[:, kb, :],
                                start=(kb == 0), stop=(kb == FB - 1))
                        nc.scalar.activation(
                            vt[:, tb, nn * 6:(nn + 1) * 6, 0:HD],
                            pv[:].rearrange("p (h d) -> p h d", d=HD),
                            AF.Identity)
                # ---- qT ----
                qT = ap.tile([P, FB, SL], f32r, tag="qT", name=f"qT{l}")
                for h1 in range(2):
                    wqh = wload(wd[l]["wq"][:, h1 * 384:(h1 + 1) * 384],
                                f"wq{l}_{h1}")
                    for m3 in range(3):
                        mb = 3 * h1 + m3
                        pq = psp.tile([P, SL], f32, tag="ps",
                                      name=f"pq{l}_{mb}")
                        for kb in range(FB):
                            nc.tensor.matmul(
                                pq[:], wqh[:, kb, m3 * P:(m3 + 1) * P],
                                slab[:, kb, W:W + SL],
                                start=(kb == 0), stop=(kb == FB - 1))
                        nc.scalar.activation(qT[:, mb, :], pq[:], AF.Identity,
                                             bias=pp[l][:, 0, mb:mb + 1],
                                             scale=0.125)
                if stage == 2 and l == 0:
                    nc.sync.dma_start(dbg_d[:, :, 0:SL], qT[:].bitcast(f32))
                    break

                # ---- kT per head-block, fused attention ----
                aT = ap.tile([P, FB, SL], f32r, tag="aT", name=f"aT{l}")
                for hb in range(FB):
                    if hb % 3 == 0:
                        wkh = wload(wd[l]["wk"][:, (hb // 3) * 384:
                                                (hb // 3 + 1) * 384],
                                    f"wk{l}_{hb // 3}")
                    kTb = ap.tile([P, SLAB], f32r, tag="kTb", bufs=2,
                                  name=f"kT{l}_{hb}")
                    for nn in range(2):
                        pk = psp.tile([P, SL], f32, tag="ps",
                                      name=f"pk{l}_{hb}_{nn}")
                        for kb in range(FB):
                            nc.tensor.matmul(
                                pk[:], wkh[:, kb, (hb % 3) * P:(hb % 3 + 1) * P],
                                slab[:, kb, nn * SL:(nn + 1) * SL],
                                start=(kb == 0), stop=(kb == FB - 1))
                        nc.scalar.activation(kTb[:, nn * SL:(nn + 1) * SL],
                                             pk[:], AF.Identity,
                                             bias=pp[l][:, 1, hb:hb + 1])
                    for ch in range(2):
                        for hh in range(2):
                            h = 2 * hb + hh
                            hp = 64 * hh
                            pv_ps = psp.tile([HD + 1, 256], f32, tag="ps",
                                             name=f"pv{l}_{hb}_{ch}_{hh}")
                            for j in range(FB):
                                st = psp.tile([P, 256], f32, tag="ps",
                                              name=f"st{l}_{hb}_{ch}_{hh}_{j}")
                                nc.tensor.matmul(
                                    st[:],
                                    kTb[hp:hp + HD,
                                        (2 * ch + j) * P:(2 * ch + j + 1) * P],
                                    qT[hp:hp + HD, hb, ch * 256:(ch + 1) * 256],
                                    start=True, stop=True)
                                ptmp = sbp.tile([P, 256], f32, tag="ptmp",
                                                name=f"pt{l}_{hb}_{ch}_{hh}_{j}")
                                nc.vector.tensor_tensor(
                                    ptmp[:], st[:], madd[:, ch, j, :],
                                    op=ALU.add)
                                PT = sbp.tile([P, 256], f32r, tag="PT", bufs=3,
                                              name=f"PT{l}_{hb}_{ch}_{hh}_{j}")
                                nc.scalar.activation(PT[:], ptmp[:], AF.Exp)
                                nc.tensor.matmul(
                                    pv_ps[:], vt[:, 2 * ch + j, h, :], PT[:],
                                    start=(j == 0), stop=(j == FB - 1))
                            rcp = sbp.tile([HD + 1, 256], f32, tag="rcp", bufs=1,
                                           name=f"rc{l}_{hb}_{ch}_{hh}")
                            nc.vector.reciprocal(rcp[HD:HD + 1, :],
                                                 pv_ps[HD:HD + 1, :])
                            rcpr = sbp.tile([HD + 1, 256], f32r, tag="rcpr", bufs=1,
                                            name=f"rr{l}_{hb}_{ch}_{hh}")
                            nc.scalar.activation(rcpr[HD:HD + 1, :],
                                                 rcp[HD:HD + 1, :], AF.Identity)
                            rb = psp.tile([HD, 256], f32, tag="ps",
                                          name=f"rb{l}_{hb}_{ch}_{hh}")
                            nc.tensor.matmul(rb[:], ones[HD:HD + 1, 0:HD],
                                             rcpr[HD:HD + 1, :],
                                             start=True, stop=True)
                            rbs = sbp.tile([HD, 256], f32, tag="rbs", bufs=1,
                                           name=f"rbs{l}_{hb}_{ch}_{hh}")
                            nc.scalar.activation(rbs[:], rb[:], AF.Copy)
                            if hh == 0:
                                nc.vector.tensor_tensor(
                                    aT[0:HD, hb, ch * 256:(ch + 1) * 256],
                                    pv_ps[0:HD, :], rbs[:], op=ALU.mult)
                            else:
                                stg = sbp.tile([HD, 256], f32r, tag="stg", bufs=1,
                                               name=f"sg{l}_{hb}_{ch}")
                                nc.vector.tensor_tensor(stg[:], pv_ps[0:HD, :],
                                                        rbs[:], op=ALU.mult)
                                nc.sync.dma_start(
                                    aT[64:128, hb, ch * 256:(ch + 1) * 256],
                                    stg[:])
                if stage == 3 and l == 0:
                    nc.sync.dma_start(dbg_d[:, :, 0:SL], aT[:].bitcast(f32))
                    break

                # ---- O-proj + residual ----
                for h1 in range(2):
                    woh = wload(wd[l]["wo"][:, h1 * 384:(h1 + 1) * 384],
                                f"wo{l}_{h1}")
                    for m3 in range(3):
                        mb = 3 * h1 + m3
                        po = psp.tile([P, SL], f32, tag="ps",
                                      name=f"po{l}_{mb}")
                        for kb in range(FB):
                            nc.tensor.matmul(
                                po[:], woh[:, kb, m3 * P:(m3 + 1) * P],
                                aT[:, kb, :],
                                start=(kb == 0), stop=(kb == FB - 1))
                        t = sbp.tile([P, SL], f32, tag="ot", name=f"ot{l}_{mb}")
                        nc.scalar.activation(t[:], po[:], AF.Identity,
                                             bias=pp[l][:, 2, mb:mb + 1])
                        nc.vector.tensor_tensor(res1[:, mb, :], t[:],
                                                xown[:, mb, :].bitcast(f32),
                                                op=ALU.add)
            if stage in (2, 3):
                break

            # ---- LN1 -> xm ----
            xm = bigp.tile([P, FB, SL], f32r, tag="xm", name=f"xm{l}")
            _ln_T(nc, sbp, psp, ones, res1,
                  lambda b, _x=xm: _x[:, b, :],
                  lambda b, _l=l: pp[_l][:, 4, b:b + 1],
                  lambda b, _l=l: pp[_l][:, 5, b:b + 1])

            # ---- FFN ----
            res2 = bigp.tile([P, FB, SL], f32r, tag="res", name=f"res2_{l}")
            with tc.tile_pool(name=f"ffn{l}", bufs=1) as fp:
                hT = fp.tile([P, FFB, SL], f32r, tag="hT", name=f"hT{l}")
                for q8 in range(8):
                    w1c = wload(wd[l]["w1"][:, q8 * 384:(q8 + 1) * 384],
                                f"w1{l}_{q8}")
                    for m3 in range(3):
                        mb = 3 * q8 + m3
                        ph = psp.tile([P, SL], f32, tag="ps",
                                      name=f"ph{l}_{mb}")
                        for kb in range(FB):
                            nc.tensor.matmul(
                                ph[:], w1c[:, kb, m3 * P:(m3 + 1) * P],
                                xm[:, kb, :],
                                start=(kb == 0), stop=(kb == FB - 1))
                        nc.scalar.activation(hT[:, mb, :], ph[:], AF.Gelu,
                                             bias=b1t[l][:, mb:mb + 1])
                # FFN2: k-major accumulation into 6 psum tiles
                f2ps = [psp.tile([P, SL], f32, tag="ps", name=f"f2{l}_{mb}")
                        for mb in range(FB)]
                for q8 in range(8):
                    w2c = wp.tile([P, 3, HID], f32r, tag="wt", name=f"w2{l}_{q8}")
                    nc.sync.dma_start(
                        w2c[:], wd[l]["w2"][q8 * 384:(q8 + 1) * 384, :]
                        .rearrange("(a p) m -> p a m", p=P))
                    for i3 in range(3):
                        for mb in range(FB):
                            nc.tensor.matmul(
                                f2ps[mb][:], w2c[:, i3, mb * P:(mb + 1) * P],
                                hT[:, 3 * q8 + i3, :],
                                start=(q8 == 0 and i3 == 0),
                                stop=(q8 == 7 and i3 == 2))
                for mb in range(FB):
                    t2 = sbp.tile([P, SL], f32, tag="ot", name=f"f2t{l}_{mb}")
                    nc.scalar.activation(t2[:], f2ps[mb][:], AF.Identity,
                                         bias=pp[l][:, 3, mb:mb + 1])
                    nc.vector.tensor_tensor(res2[:, mb, :], t2[:],
                                            xm[:, mb, :].bitcast(f32),
                                            op=ALU.add)

            # ---- LN2 ----
            if l == 0:
                nxt = bigp.tile([P, FB, SLAB], f32r, tag="slab", bufs=2,
                                name="slab2")
                _ln_T(nc, sbp, psp, ones, res2,
                      lambda b, _n=nxt: _n[:, b, W:W + SL],
                      lambda b, _l=l: pp[_l][:, 6, b:b + 1],
                      lambda b, _l=l: pp[_l][:, 7, b:b + 1])
                if stage == 4:
                    nc.sync.dma_start(dbg_d[:, :, 0:SL],
                                      nxt[:, :, W:W + SL].bitcast(f32))
                    break
                # ---- AllGather + halo gathers ----
                ag_in = dramp.tile([2, FB, P, 256], f32)
                ag_out = dramp.tile([NCORES, 2, FB, P, 256], f32,
                                    addr_space="Shared")
                nc.sync.dma_start(ag_in[0].rearrange("b p q -> p b q"),
                                  nxt[:, :, W:2 * W].bitcast(f32))
                nc.sync.dma_start(ag_in[1].rearrange("b p q -> p b q"),
                                  nxt[:, :, 2 * W:3 * W].bitcast(f32))
                nc.gpsimd.collective_compute(
                    "AllGather", ALU.bypass,
                    replica_groups=[list(range(NCORES))],
                    ins=[ag_in.opt()], outs=[ag_out.opt()])
                ag_flat = ag_out.rearrange("c h b p q -> (c h b p) q")
                for g in range(12):
                    j = g % FB
                    out_ap = (nxt[:, j, 0:W] if g < FB
                              else nxt[:, j, 3 * W:SLAB])
                    nc.gpsimd.indirect_dma_start(
                        out=out_ap, out_offset=None, in_=ag_flat,
                        in_offset=bass.IndirectOffsetOnAxis(
                            ap=hidx[:, g:g + 1], axis=0))
                slab = nxt
                if stage == 6:
                    nc.sync.dma_start(dbg_d[:], nxt[:].bitcast(f32))
                    break
                if stage == 7:
                    for half in range(2):
                        for b in range(FB):
                            agt = sbp.tile([P, 256], f32, tag="ptmp",
                                           name=f"agd{half}_{b}")
                            nc.sync.dma_start(agt[:], ag_out[3, half, b])
                            nc.sync.dma_start(
                                dbg_d[:, b, half * 256:(half + 1) * 256], agt[:])
                            agi = sbp.tile([P, 256], f32, tag="ptmp",
                                           name=f"agi{half}_{b}")
                            nc.sync.dma_start(agi[:], ag_in[half, b])
                            nc.sync.dma_start(
                                dbg_d[:, b, 512 + half * 256:512 + (half + 1) * 256],
                                agi[:])
                    break
            else:
                with tc.tile_pool(name="outp", bufs=2) as op_:
                    def _mkdst(b, _p=op_):
                        t = _p.tile([P, SL], f32, tag="xo", name=f"xo{b}")
                        return t
                    dsts = [_mkdst(b) for b in range(FB)]
                    _ln_T(nc, sbp, psp, ones, res2,
                          lambda b, _d=dsts: _d[b][:],
                          lambda b, _l=l: pp[_l][:, 6, b:b + 1],
                          lambda b, _l=l: pp[_l][:, 7, b:b + 1])
                    for b in range(FB):
                        nc.sync.dma_start(xout_d[b], dsts[b][:])

        if stage != 5:
            # touch every input + write xout so the NEFF keeps all I/O bound
            scr = constp.tile([P, 1024], f32, name="scratch")
            for l in range(L):
                for k in ("wq", "wk", "wv", "wo", "w1", "w2"):
                    nc.sync.dma_start(scr[:, 0:P],
                                      wd[l][k][0:P, 0:P].bitcast(f32))
            nc.sync.dma_start(xout_d[:],
                              slab[:, :, 0:SL].bitcast(f32)
                              .rearrange("p b t -> b p t"))

    nc.compile()
    _cache[stage] = nc
    return nc


# ---------------------------------------------------------------------------
def prep_inputs(inputs):
    ip = np.asarray(inputs["ip"]).astype(np.int64)
    mask = np.asarray(inputs["mask"]).astype(np.int32)
    we = np.asarray(inputs["word_emb"], dtype=np.float32)
    pe = np.asarray(inputs["pos_emb"], dtype=np.float32)
    te = np.asarray(inputs["type_emb"], dtype=np.float32)
    m = mask[0]
    pos_ids = (np.cumsum(m) * m + 1).astype(np.int64)

    def pack(v):  # [768] -> [128, 6]
        return np.ascontiguousarray(np.asarray(v, np.float32).reshape(FB, P).T)

    lnemb = np.ascontiguousarray(
        np.stack([pack(inputs["ln_emb_g"]), pack(inputs["ln_emb_b"])], axis=1))

    # shared per-layer tensors
    shared = {}
    for l in range(L):
        Wo = np.asarray(inputs["Wo"][l], np.float32)
        bv = np.asarray(inputs["bv"][l], np.float32)
        boeff = Wo.T @ bv + np.asarray(inputs["bo"][l], np.float32)
        ppk = np.ascontiguousarray(np.stack([
            pack(np.asarray(inputs["bq"][l], np.float32) * 0.125),
            pack(inputs["bk"][l]), pack(boeff), pack(inputs["b2"][l]),
            pack(inputs["ln1_g"][l]), pack(inputs["ln1_b"][l]),
            pack(inputs["ln2_g"][l]), pack(inputs["ln2_b"][l])], axis=1))
        shared[f"wq{l}"] = np.asarray(inputs["Wq"][l], np.float32)
        shared[f"wk{l}"] = np.asarray(inputs["Wk"][l], np.float32)
        shared[f"wv{l}"] = np.asarray(inputs["Wv"][l], np.float32)
        shared[f"wo{l}"] = Wo
        shared[f"w1{l}"] = np.asarray(inputs["W1"][l], np.float32)
        shared[f"w2{l}"] = np.asarray(inputs["W2"][l], np.float32)
        shared[f"pp{l}"] = ppk
        shared[f"b1_{l}"] = np.ascontiguousarray(
            np.asarray(inputs["b1"][l], np.float32).reshape(FFB, P).T)
    shared["onesr"] = np.ones((P, P), np.float32)
    shared["lnemb"] = lnemb

    in_maps = []
    pr = np.arange(P)
    for c in range(NCORES):
        t0 = c * SL - W
        tt = np.clip(np.arange(t0, t0 + SLAB), 0, S - 1)
        xemb = we[ip[0, tt]] + pe[pos_ids[tt]] + te[0]

        madd = np.full((P, 2, FB, 256), NEG, np.float32)
        pj = pr[:, None, None]
        jj = np.arange(FB)[None, :, None]
        qq = np.arange(256)[None, None, :]
        rel = 128 * jj + pj - qq
        band = (rel >= 0) & (rel <= 2 * W)
        for ch in range(2):
            kg = c * SL + 256 * ch - W + 128 * jj + pj + 0 * qq
            valid = (kg >= 0) & (kg < S) & (m[np.clip(kg, 0, S - 1)] == 1)
            madd[:, ch][band & valid] = 0.0

        hidx = np.zeros((P, 12), np.int32)
        for g in range(12):
            j = g % FB
            cc, half = (max(c - 1, 0), 1) if g < FB else (min(c + 1, NCORES - 1), 0)
            hidx[:, g] = ((cc * 2 + half) * FB + j) * P + pr

        im = dict(shared)
        im["xemb"] = np.ascontiguousarray(xemb, np.float32)
        im["madd"] = madd.astype(ml_dtypes.bfloat16)
        im["hidx"] = hidx
        in_maps.append(im)
    return in_maps


def kernel(**inputs):
    stage = int(os.environ.get("KSTAGE", "5"))
    nc = build(stage)
    in_maps = prep_inputs(inputs)
    res = run_bass_kernel_spmd(nc, in_maps, list(range(NCORES)))
    outs = []
    for c in range(NCORES):
        xo = res.results[c]["xout"]  # [6, 128, 512]
        outs.append(np.transpose(xo, (2, 0, 1)).reshape(SL, HID))
    return np.concatenate(outs, axis=0)[None].astype(np.float32)



# revision 25
# speedup vs baseline: 1.5354x; 1.5354x over previous
"""Longformer-style 2-layer encoder (S=4096, HID=768, sliding window W=256)
on 8 Trainium2 NeuronCores.

Sharding: sequence-parallel. Core c owns tokens [512c, 512c+512). Layer-1
halos are computed locally from embeddings over a 1024-token slab; layer-2
halos arrive via a bf16 AllGather of layer-1 outputs + two wide per-core
indirect gathers, overlapped with layer-2 own-token QKV compute.

Weights are bf16 (half the DMA, fast weight load on PE); activations in the
attention path are bf16; the residual stream stays f32r. Attention scores
are exp'd directly from PSUM (no additive mask); the band/validity mask is
applied as a 0/1 bf16 multiply on DVE (2x mode). Softmax denominators ride
the PV matmul as a 65th all-ones column of V; reciprocal via the single-op
DVE reciprocal_approx_fast; normalization = K=1 broadcast matmul + DVE
multiply. LayerNorm over the feature (partition) axis uses ones-column
matmuls for stats and Ln/Exp on ACT for rsqrt (stays in the
natural_log_exp table set; only Gelu switches sets).
"""
import os
import sys
import types
from contextlib import ExitStack

import ml_dtypes
import numpy as np

# --- optional NTFF profiling shim (antenv.axon_hooks missing in image) ----
try:
    import antenv
    if 'antenv.axon_hooks' not in sys.modules:
        _mod = types.ModuleType('antenv.axon_hooks')
        _hook = [None]
        _mod.set_axon_ntff_profile_hook = lambda h: _hook.__setitem__(0, h)
        _mod.get_axon_ntff_profile_hook = lambda: _hook[0]
        sys.modules['antenv.axon_hooks'] = _mod
        antenv.axon_hooks = _mod
        try:
            from trn_agent_boot.trn_boot import _ntff_profile_via_ctypes
            _mod.set_axon_ntff_profile_hook(
                _ntff_profile_via_ctypes('/opt/axon/libaxon_pjrt.so'))
        except Exception:
            pass
except Exception:
    pass

import concourse.bass as bass
import concourse.mybir as mybir
import concourse.tile as tile
from concourse import bacc
from concourse.bass_utils import run_bass_kernel_spmd
from concourse.masks import make_identity

f32 = mybir.dt.float32
f32r = mybir.dt.float32r
bf16 = mybir.dt.bfloat16
i32 = mybir.dt.int32
AF = mybir.ActivationFunctionType
ALU = mybir.AluOpType

NCORES = 8
P = 128
S, HID, NH, HD, FF, L = 4096, 768, 12, 64, 3072, 2
W = 256
SL = 512
SLAB = 1024
FB = HID // P     # 6
TB = SLAB // P    # 8
FFB = FF // P     # 24
EPS = 1e-5

_cache = {}


def _ln_T(nc, tc, stk, sbp, psL, ones, negr, src, dst_fn, g_ap, b_ap):
    """LayerNorm over the 768-feature partition axis of transposed
    activations src [128, FB, 512] (f32r). dst_fn(b) -> out AP block."""
    rows = psL.tile([P, SL], f32, tag="lnrow", bufs=2, name="rows")
    rowq = psL.tile([P, SL], f32, tag="lnrow", bufs=2, name="rowq")
    for b in range(FB):
        sq = sbp.tile([P, SL], f32r, tag="lnsq", name=f"lnsq{b}")
        nc.scalar.activation(sq[:], src[:, b, :].bitcast(f32), AF.Square)
        nc.tensor.matmul(rows[0:1, :], ones[:, 0:1], src[:, b, :],
                         start=(b == 0), stop=(b == FB - 1))
        nc.tensor.matmul(rowq[0:1, :], ones[:, 0:1], sq[:],
                         start=(b == 0), stop=(b == FB - 1))
    stat = sbp.tile([1, 2, SL], f32r, tag="lnstat", bufs=1, name="stat")
    # stat[0,0] = mean, stat[0,1] = E[x^2] + eps
    nc.vector.tensor_scalar_mul(stat[:, 0, :], rows[0:1, :], 1.0 / HID)
    nc.vector.tensor_scalar(stat[:, 1, :], rowq[0:1, :], 1.0 / HID, EPS,
                            op0=ALU.mult, op1=ALU.add)
    var = sbp.tile([1, SL], f32, tag="lnvar", bufs=1, name="var")
    nc.vector.tensor_tensor(var[:], stat[:, 0, :].bitcast(f32),
                            stat[:, 0, :].bitcast(f32), op=ALU.mult)
    nc.vector.tensor_tensor(var[:], stat[:, 1, :].bitcast(f32), var[:],
                            op=ALU.subtract)
    lv = sbp.tile([1, SL], f32, tag="lnlv", bufs=1, name="lv")
    nc.scalar.activation(lv[:], var[:], AF.Ln)
    c1 = sbp.tile([1, SL], f32r, tag="lnc1", bufs=1, name="c1")
    nc.scalar.activation(c1[:], lv[:], AF.Exp, scale=-0.5)   # rstd
    # every DVE apply transitively depends on all earlier DVE row ops
    # (through the ACT/PE chain) so the static schedule cannot deadlock.
    mb = psL.tile([P, SL], f32, tag="lnb", bufs=2, name="mb")
    c1b = psL.tile([P, SL], f32, tag="lnb", bufs=2, name="c1b")
    nc.tensor.matmul(mb[:], ones[0:1, :], stat[:, 0, :], start=True, stop=True)
    nc.tensor.matmul(c1b[:], ones[0:1, :], c1[:], start=True, stop=True)
    for b in range(FB):
        t = sbp.tile([P, SL], f32, tag="lnap", name=f"lnap{b}")
        nc.vector.tensor_tensor(t[:], src[:, b, :].bitcast(f32), mb[:],
                                op=ALU.subtract)
        nc.vector.tensor_tensor(t[:], t[:], c1b[:], op=ALU.mult)
        nc.scalar.activation(dst_fn(b), t[:], AF.Identity,
                             bias=b_ap(b), scale=g_ap(b))


def build(stage=5):
    if stage in _cache:
        return _cache[stage]
    nc = bacc.Bacc("TRN2", target_bir_lowering=False, debug=False,
                   num_devices=NCORES)

    xemb_d = nc.dram_tensor("xemb", [SLAB, HID], f32, kind="ExternalInput")
    lnemb_d = nc.dram_tensor("lnemb", [P, 2, FB], f32, kind="ExternalInput")
    bmask_d = nc.dram_tensor("bmask", [P, 2, FB, 256], bf16,
                             kind="ExternalInput")
    hidx_d = nc.dram_tensor("hidx", [P, 2], i32, kind="ExternalInput")
    ones_d = nc.dram_tensor("onesr", [P, P], f32r, kind="ExternalInput")
    negr_d = nc.dram_tensor("negr", [P, P], f32r, kind="ExternalInput")
    onesb_d = nc.dram_tensor("onesb", [P, P], bf16, kind="ExternalInput")
    wd, ppd, b1d = [], [], []
    for l in range(L):
        wd.append({k: nc.dram_tensor(f"{k}{l}", shp, bf16, kind="ExternalInput")
                   for k, shp in [("wq", [HID, HID]), ("wk", [HID, HID]),
                                  ("wv", [HID, HID]), ("wo", [HID, HID]),
                                  ("w1", [HID, FF]), ("w2", [FF, HID])]})
        ppd.append(nc.dram_tensor(f"pp{l}", [P, 8, FB], f32, kind="ExternalInput"))
        b1d.append(nc.dram_tensor(f"b1_{l}", [P, FFB], f32, kind="ExternalInput"))

    xout_d = nc.dram_tensor("xout", [FB, P, SL], f32, kind="ExternalOutput")
    dbg_d = (nc.dram_tensor("dbg", [P, FB, SLAB], f32, kind="ExternalOutput")
             if stage != 5 else None)

    with tile.TileContext(nc) as tc, ExitStack() as top:
        constp = top.enter_context(tc.tile_pool(name="const", bufs=1))
        sbp = top.enter_context(tc.tile_pool(name="sb", bufs=2))
        bigp = top.enter_context(tc.tile_pool(name="big", bufs=1))
        wp = top.enter_context(tc.tile_pool(name="wp", bufs=3))
        dramp = top.enter_context(tc.tile_pool(name="dram", bufs=1, space="DRAM"))

        ident = constp.tile([P, P], f32)
        make_identity(nc, ident[:])
        lnemb = constp.tile([P, 2, FB], f32)
        nc.sync.dma_start(lnemb[:], lnemb_d[:])
        ones = constp.tile([P, P], f32r)
        nc.sync.dma_start(ones[:], ones_d[:])
        negr = constp.tile([P, P], f32r)
        nc.sync.dma_start(negr[:], negr_d[:])
        bmask = constp.tile([P, 2, FB, 256], bf16)
        nc.sync.dma_start(bmask[:], bmask_d[:])
        hidx = constp.tile([P, 2], i32)
        nc.sync.dma_start(hidx[:], hidx_d[:])
        pp, b1t = [], []
        for l in range(L):
            ppt = constp.tile([P, 8, FB], f32, name=f"pp{l}")
            nc.sync.dma_start(ppt[:], ppd[l][:])
            pp.append(ppt)
            b1 = constp.tile([P, FFB], f32, name=f"b1_{l}")
            nc.sync.dma_start(b1[:], b1d[l][:])
            b1t.append(b1)

        def wload(dram_ap, name):
            t = wp.tile([P, FB, 384], bf16, tag="wt", name=name)
            nc.sync.dma_start(t[:], dram_ap.rearrange("(kb kp) m -> kp kb m",
                                                      kp=P))
            return t

        # ================= embedding: natural LN + transpose ==============
        # slab_bf: full 1024-token slab, bf16 (matmul operand side)
        # slab_own: own 512 tokens, f32r (residual base / LN side)
        slab_bf = bigp.tile([P, FB, SLAB], bf16, tag="slabbf", name="slabbf")
        slab_own = bigp.tile([P, FB, SL], f32r, tag="slab", name="slab1")
        with tc.tile_pool(name="embp", bufs=2) as ep, \
             tc.tile_pool(name="psE", bufs=4, space="PSUM") as pse:
            for tb in range(TB):
                xe = ep.tile([P, HID], f32, tag="xe", name=f"xe{tb}")
                nc.sync.dma_start(xe[:], xemb_d[tb * P:(tb + 1) * P, :])
                nm = ep.tile([P, 1], f32, tag="enm", name=f"nm{tb}")
                nc.vector.reduce_sum(out=nm[:], in_=xe[:],
                                     axis=mybir.AxisListType.X)
                nc.vector.tensor_scalar_mul(nm[:], nm[:], -1.0 / HID)
                xc = ep.tile([P, HID], f32, tag="exc", bufs=1, name=f"xc{tb}")
                nc.scalar.activation(xc[:], xe[:], AF.Identity, bias=nm[:, 0:1])
                sqs = ep.tile([P, HID], f32, tag="esq", bufs=1, name=f"sqs{tb}")
                var = ep.tile([P, 1], f32, tag="evar", name=f"var{tb}")
                rstd = ep.tile([P, 1], f32, tag="ers", name=f"rstd{tb}")
                nc.scalar.activation(sqs[:], xc[:], AF.Square)
                nc.vector.reduce_sum(out=var[:], in_=sqs[:],
                                     axis=mybir.AxisListType.X)
                nc.vector.tensor_scalar(var[:], var[:], 1.0 / HID, EPS,
                                        op0=ALU.mult, op1=ALU.add)
                nc.scalar.activation(var[:], var[:], AF.Ln)
                nc.scalar.activation(rstd[:], var[:], AF.Exp, scale=-0.5)
                xn = ep.tile([P, HID], f32, tag="exn", name=f"xn{tb}")
                nc.scalar.activation(xn[:], xc[:], AF.Identity,
                                     scale=rstd[:, 0:1])
                for b in range(FB):
                    tp = pse.tile([P, P], f32, tag="tp", name=f"tp{tb}_{b}")
                    nc.tensor.transpose(tp[:], xn[:, b * P:(b + 1) * P],
                                        ident[:])
                    nc.scalar.activation(
                        slab_bf[:, b, tb * P:(tb + 1) * P], tp[:], AF.Identity,
                        bias=lnemb[:, 1, b:b + 1], scale=lnemb[:, 0, b:b + 1])
                    if 2 <= tb < 6:
                        nc.scalar.activation(
                            slab_own[:, b, (tb - 2) * P:(tb - 1) * P], tp[:],
                            AF.Identity, bias=lnemb[:, 1, b:b + 1],
                            scale=lnemb[:, 0, b:b + 1])

        if stage == 1:
            for b in range(FB):
                sf = sbp.tile([P, SLAB], f32, tag="dump1", name=f"sf{b}")
                nc.scalar.activation(sf[:], slab_bf[:, b, :], AF.Identity)
                nc.sync.dma_start(dbg_d[:, b, :], sf[:])

        # persistent across-layer tiles
        s2own = bigp.tile([P, FB, SL], f32r, tag="s2own", name="s2own")
        haloL = bigp.tile([P, FB, 256], bf16, tag="haloL", name="haloL")
        haloR = bigp.tile([P, FB, 256], bf16, tag="haloR", name="haloR")
        ag_in = dramp.tile([2, P, FB, 256], bf16)
        ag_out = dramp.tile([NCORES, 2, P, FB, 256], bf16, addr_space="Shared")

        # ======================= transformer layers =======================
        for l in range(L):
            if stage <= 1:
                break

            # per-token-block lhsT accessor (V-proj stationary side), and
            # K-proj rhs ranges per 512-half, and own-x accessor.
            if l == 0:
                def blk(kb, t, _s=slab_bf):
                    return _s[:, kb, t * P:(t + 1) * P]
                kranges = [[(0, 512, lambda kb, _s=slab_bf: _s[:, kb, 0:512])],
                           [(0, 512, lambda kb, _s=slab_bf: _s[:, kb, 512:1024])]]
                def xbf(kb, _s=slab_bf):
                    return _s[:, kb, W:W + SL]
                def xown(b, _s=slab_own):
                    return _s[:, b, :]
            else:
                def blk(kb, t, _o=ec, _hl=haloL, _hr=haloR):
                    if t < 2:
                        return _hl[:, kb, t * P:(t + 1) * P]
                    if t >= 6:
                        return _hr[:, kb, (t - 6) * P:(t - 5) * P]
                    return _o[:, kb, (t - 2) * P:(t - 1) * P]
                kranges = [[(0, 256, lambda kb, _h=haloL: _h[:, kb, :]),
                            (256, 512, lambda kb, _o=ec: _o[:, kb, 0:256])],
                           [(0, 256, lambda kb, _o=ec: _o[:, kb, 256:512]),
                            (256, 512, lambda kb, _h=haloR: _h[:, kb, :])]]
                def xbf(kb, _o=ec):
                    return _o[:, kb, :]
                def xown(b, _o=s2own):
                    return _o[:, b, :]

            res1 = bigp.tile([P, FB, SL], f32r, tag="res", bufs=1,
                             name=f"res1_{l}")
            with tc.tile_pool(name=f"attn{l}", bufs=1) as ap, \
                 tc.tile_pool(name=f"psA{l}", bufs=1, space="PSUM") as psA:
                # ---- kT [feat, tok] bf16, per head-block ----
                kT = ap.tile([P, FB, SLAB], bf16, tag="kT", name=f"kT{l}")
                for nn in range(2):
                    wkh = wload(wd[l]["wk"][:, nn * 384:(nn + 1) * 384],
                                f"wk{l}_{nn}")
                    for m3 in range(3):
                        hb = 3 * nn + m3
                        for half in range(2):
                            pk = psA.tile([P, SL], f32, tag="proj", bufs=3,
                                          name=f"pk{l}_{hb}_{half}")
                            for (c0, c1, rap) in kranges[half]:
                                for kb in range(FB):
                                    nc.tensor.matmul(
                                        pk[:, c0:c1],
                                        wkh[:, kb, m3 * P:(m3 + 1) * P],
                                        rap(kb),
                                        start=(kb == 0), stop=(kb == FB - 1))
                            nc.scalar.activation(
                                kT[:, hb, half * SL:(half + 1) * SL], pk[:],
                                AF.Identity, bias=pp[l][:, 1, hb:hb + 1])
                # ---- qT ----
                qT = ap.tile([P, FB, SL], bf16, tag="qT", name=f"qT{l}")
                for nn in range(2):
                    wqh = wload(wd[l]["wq"][:, nn * 384:(nn + 1) * 384],
                                f"wq{l}_{nn}")
                    for m3 in range(3):
                        mb = 3 * nn + m3
                        pq = psA.tile([P, SL], f32, tag="proj", bufs=3,
                                      name=f"pq{l}_{mb}")
                        for kb in range(FB):
                            nc.tensor.matmul(
                                pq[:], wqh[:, kb, m3 * P:(m3 + 1) * P],
                                xbf(kb), start=(kb == 0), stop=(kb == FB - 1))
                        nc.scalar.activation(qT[:, mb, :], pq[:], AF.Identity,
                                             bias=pp[l][:, 0, mb:mb + 1],
                                             scale=0.125)
                if stage == 2 and l == 0:
                    for b in range(FB):
                        qf = sbp.tile([P, SL], f32, tag="dump", name=f"qf{b}")
                        nc.scalar.activation(qf[:], qT[:, b, :], AF.Identity)
                        nc.sync.dma_start(dbg_d[:, b, 0:SL], qf[:])
                    break
                # ---- v natural (ones-augmented) bf16 ----
                vt = ap.tile([P, TB, NH, HD + 1], bf16, tag="vt", name=f"vt{l}")
                nc.sync.dma_start(
                    vt[:, :, :, HD:HD + 1],
                    onesb_d[:, 0:TB * NH].rearrange("p (a b c) -> p a b c",
                                                    a=TB, b=NH))
                for nn in range(2):
                    wvh = wload(wd[l]["wv"][:, nn * 384:(nn + 1) * 384],
                                f"wv{l}_{nn}")
                    for t in range(TB):
                        pv = psA.tile([P, 384], f32, tag="proj", bufs=3,
                                      name=f"pvv{l}_{nn}_{t}")
                        for kb in range(FB):
                            nc.tensor.matmul(
                                pv[:], blk(kb, t), wvh[:, kb, :],
                                start=(kb == 0), stop=(kb == FB - 1))
                        nc.vector.tensor_copy(
                            vt[:, t, nn * 6:(nn + 1) * 6, 0:HD],
                            pv[:].rearrange("p (h d) -> p h d", d=HD))

                # ---- fused banded attention ----
                aT = ap.tile([P, FB, SL], bf16, tag="aT", name=f"aT{l}")
                dbg_refs = {}
                for hb in range(FB if stage != 8 else 1):
                    pvb = [psA.tile([P, SL], f32, tag="pv", bufs=2,
                                    name=f"pv{l}_{hb}_{hh}",
                                    padded_shape=[P, SL])
                           for hh in range(2)]
                    for ch in range(2):
                        for hh in range(2):
                            h = 2 * hb + hh
                            hp = 64 * hh
                            PT = ap.tile([P, FB, 256], bf16, tag="PT", bufs=3,
                                         name=f"PT{l}_{hb}_{ch}_{hh}")
                            if stage == 8 and hb == 0 and ch == 0 and hh == 0:
                                dbg_refs["PT"] = PT
                            for b in range(3):
                                sc = psA.tile([P, SL], f32, tag="sc", bufs=3,
                                              name=f"sc{l}_{hb}_{ch}_{hh}_{b}")
                                for q in range(2):
                                    j = 2 * b + q
                                    nc.tensor.matmul(
                                        sc[:, q * 256:(q + 1) * 256],
                                        kT[hp:hp + HD, hb,
                                           (2 * ch + j) * P:(2 * ch + j + 1) * P],
                                        qT[hp:hp + HD, hb,
                                           ch * 256:(ch + 1) * 256],
                                        start=True, stop=True)
                                nc.scalar.activation(
                                    PT[:, 2 * b:2 * b + 2, :]
                                    .rearrange("p a q -> p (a q)"),
                                    sc[:], AF.Exp)
                                nc.vector.tensor_tensor(
                                    PT[:, 2 * b:2 * b + 2, :],
                                    PT[:, 2 * b:2 * b + 2, :],
                                    bmask[:, ch, 2 * b:2 * b + 2, :],
                                    op=ALU.mult)
                            for j in range(FB):
                                nc.tensor.matmul(
                                    pvb[hh][0:HD + 1,
                                            ch * 256:(ch + 1) * 256],
                                    vt[:, 2 * ch + j, h, :], PT[:, j, :],
                                    start=(j == 0), stop=(j == FB - 1),
                                    skip_group_check=True)
                    for hh in range(2):
                        rcp = ap.tile([HD + 1, SL], f32, tag="rcp", bufs=2,
                                      name=f"rc{l}_{hb}_{hh}")
                        nc.scalar.activation(rcp[HD:HD + 1, :],
                                             pvb[hh][HD:HD + 1, :], AF.Ln)
                        rcpr = ap.tile([HD + 1, SL], f32r, tag="rcpr", bufs=2,
                                       name=f"rr{l}_{hb}_{hh}")
                        nc.scalar.activation(rcpr[HD:HD + 1, :],
                                             rcp[HD:HD + 1, :], AF.Exp,
                                             scale=-1.0)
                        rb = psA.tile([HD, SL], f32, tag="proj", bufs=3,
                                      name=f"rb{l}_{hb}_{hh}")
                        nc.tensor.matmul(rb[:], ones[HD:HD + 1, 0:HD],
                                         rcpr[HD:HD + 1, :],
                                         start=True, stop=True)
                        rbs = sbp.tile([HD, SL], f32, tag="rbs", bufs=2,
                                       name=f"rbs{l}_{hb}_{hh}")
                        nc.vector.tensor_copy(rbs[:], rb[:])
                        if stage == 8 and hb == 0 and hh == 0:
                            dbg_refs["pvb"] = pvb[0]
                            dbg_refs["rbs"] = rbs
                        if hh == 0:
                            nc.vector.tensor_tensor(
                                aT[0:HD, hb, :], pvb[0][0:HD, :], rbs[:],
                                op=ALU.mult)
                        else:
                            stg = sbp.tile([HD, SL], bf16, tag="stg", bufs=2,
                                           name=f"sg{l}_{hb}")
                            nc.vector.tensor_tensor(stg[:], pvb[1][0:HD, :],
                                                    rbs[:], op=ALU.mult)
                            nc.sync.dma_start(aT[64:128, hb, :], stg[:])
                if stage == 3 and l == 0:
                    for b in range(FB):
                        af = sbp.tile([P, SL], f32, tag="dump", name=f"af{b}")
                        nc.scalar.activation(af[:], aT[:, b, :], AF.Identity)
                        nc.sync.dma_start(dbg_d[:, b, 0:SL], af[:])
                    break
                if stage == 8 and l == 0:
                    ptf = sbp.tile([P, FB * 256], f32, tag="dump8", bufs=1,
                                   name="ptf")
                    nc.scalar.activation(
                        ptf[:], dbg_refs["PT"].rearrange("p a q -> p (a q)"),
                        AF.Identity)
                    nc.sync.dma_start(dbg_d[:, 0, :], ptf[:, 0:1024])
                    nc.sync.dma_start(dbg_d[:, 1, 0:512], ptf[:, 1024:1536])
                    pvf = sbp.tile([P, SL], f32, tag="dump8b", bufs=1,
                                   name="pvf")
                    nc.vector.tensor_copy(pvf[:], dbg_refs["pvb"][:])
                    nc.sync.dma_start(dbg_d[:, 2, 0:SL], pvf[:])
                    nc.sync.dma_start(dbg_d[0:HD, 3, 0:SL], dbg_refs["rbs"][:])
                    atf = sbp.tile([P, SL], f32, tag="dump8c", bufs=1,
                                   name="atf")
                    nc.scalar.activation(atf[:], aT[:, 0, :], AF.Identity)
                    nc.sync.dma_start(dbg_d[:, 4, 0:SL], atf[:])
                    break

                # ---- O-proj + residual ----
                for nn in range(2):
                    woh = wload(wd[l]["wo"][:, nn * 384:(nn + 1) * 384],
                                f"wo{l}_{nn}")
                    for m3 in range(3):
                        mb = 3 * nn + m3
                        po = psA.tile([P, SL], f32, tag="proj", bufs=3,
                                      name=f"po{l}_{mb}")
                        for kb in range(FB):
                            nc.tensor.matmul(
                                po[:], woh[:, kb, m3 * P:(m3 + 1) * P],
                                aT[:, kb, :],
                                start=(kb == 0), stop=(kb == FB - 1))
                        t = sbp.tile([P, SL], f32, tag="ot", name=f"ot{l}_{mb}")
                        nc.scalar.activation(t[:], po[:], AF.Identity,
                                             bias=pp[l][:, 2, mb:mb + 1])
                        nc.vector.tensor_tensor(res1[:, mb, :], t[:],
                                                xown(mb).bitcast(f32),
                                                op=ALU.add)
            if stage in (2, 3):
                break

            # ---- LN1 -> xm (f32r for residual/LN2) + xm_bf (matmul rhs) ----
            xm = bigp.tile([P, FB, SL], f32r, tag="xm", bufs=1, name=f"xm{l}")
            xm_bf = bigp.tile([P, FB, SL], bf16, tag="xmbf", bufs=1,
                              name=f"xmbf{l}")
            with tc.tile_pool(name=f"psL1_{l}", bufs=1, space="PSUM") as psL:
                _ln_T(nc, tc, None, sbp, psL, ones, negr, res1,
                      lambda b, _x=xm: _x[:, b, :],
                      lambda b, _l=l: pp[_l][:, 4, b:b + 1],
                      lambda b, _l=l: pp[_l][:, 5, b:b + 1])
            for b in range(FB):
                nc.vector.tensor_copy(xm_bf[:, b, :], xm[:, b, :].bitcast(f32))

            # ---- FFN ----
            res2 = bigp.tile([P, FB, SL], f32r, tag="res", bufs=1,
                             name=f"res2_{l}")
            with tc.tile_pool(name=f"ffn{l}", bufs=1) as fp, \
                 tc.tile_pool(name=f"psF{l}", bufs=1, space="PSUM") as psF:
                hT = fp.tile([P, FFB, SL], bf16, tag="hT", name=f"hT{l}")
                f2ps = [psF.tile([P, SL], f32, tag="f2", bufs=6,
                                 name=f"f2{l}_{mb}") for mb in range(FB)]
                for q8 in range(8):
                    w1c = wload(wd[l]["w1"][:, q8 * 384:(q8 + 1) * 384],
                                f"w1{l}_{q8}")
                    for m3 in range(3):
                        mb = 3 * q8 + m3
                        ph = psF.tile([P, SL], f32, tag="ph", bufs=2,
                                      name=f"ph{l}_{mb}")
                        for kb in range(FB):
                            nc.tensor.matmul(
                                ph[:], w1c[:, kb, m3 * P:(m3 + 1) * P],
                                xm_bf[:, kb, :],
                                start=(kb == 0), stop=(kb == FB - 1))
                        nc.scalar.activation(hT[:, mb, :], ph[:], AF.Gelu,
                                             bias=b1t[l][:, mb:mb + 1])
                    # FFN2 partial: k-major accumulation into 6 psum tiles
                    w2c = wp.tile([P, 3, HID], bf16, tag="w2", name=f"w2{l}_{q8}")
                    nc.sync.dma_start(
                        w2c[:], wd[l]["w2"][q8 * 384:(q8 + 1) * 384, :]
                        .rearrange("(a p) m -> p a m", p=P))
                    for i3 in range(3):
                        for mb in range(FB):
                            nc.tensor.matmul(
                                f2ps[mb][:], w2c[:, i3, mb * P:(mb + 1) * P],
                                hT[:, 3 * q8 + i3, :],
                                start=(q8 == 0 and i3 == 0),
                                stop=(q8 == 7 and i3 == 2))
                for mb in range(FB):
                    t2 = sbp.tile([P, SL], f32, tag="ot", name=f"f2t{l}_{mb}")
                    nc.scalar.activation(t2[:], f2ps[mb][:], AF.Identity,
                                         bias=pp[l][:, 3, mb:mb + 1])
                    nc.vector.tensor_tensor(res2[:, mb, :], t2[:],
                                            xm[:, mb, :].bitcast(f32),
                                            op=ALU.add)

            # ---- LN2 ----
            if l == 0:
                ec = bigp.tile([P, FB, SL], bf16, tag="ec", name="ec")
                with tc.tile_pool(name=f"psL2_{l}", bufs=1, space="PSUM") as psL:
                    def dst2(b, _o=s2own, _e=ec, _nc=nc, _pp=pp, _l=l):
                        return _o[:, b, :]
                    _ln_T(nc, tc, None, sbp, psL, ones, negr, res2, dst2,
                          lambda b, _l=l: pp[_l][:, 6, b:b + 1],
                          lambda b, _l=l: pp[_l][:, 7, b:b + 1])
                    # bf16 copy for the halo exchange push
                    for b in range(FB):
                        nc.vector.tensor_copy(ec[:, b, :],
                                              s2own[:, b, :].bitcast(f32))
                if stage == 4:
                    for b in range(FB):
                        nc.sync.dma_start(dbg_d[:, b, 0:SL],
                                          s2own[:, b, :].bitcast(f32))
                    break
                # ---- AllGather of own halves (bf16) + wide halo gathers ----
                for h in range(2):
                    nc.sync.dma_start(ag_in[h],
                                      ec[:, :, h * 256:(h + 1) * 256])
                nc.gpsimd.collective_compute(
                    "AllGather", ALU.bypass,
                    replica_groups=[list(range(NCORES))],
                    ins=[ag_in.opt()], outs=[ag_out.opt()])
                ag_flat = ag_out.rearrange("c h p b q -> (c h p) (b q)")
                nc.gpsimd.indirect_dma_start(
                    out=haloL.rearrange("p b q -> p (b q)"), out_offset=None,
                    in_=ag_flat,
                    in_offset=bass.IndirectOffsetOnAxis(ap=hidx[:, 0:1], axis=0))
                nc.gpsimd.indirect_dma_start(
                    out=haloR.rearrange("p b q -> p (b q)"), out_offset=None,
                    in_=ag_flat,
                    in_offset=bass.IndirectOffsetOnAxis(ap=hidx[:, 1:2], axis=0))
                if stage == 6:
                    for b in range(FB):
                        hf = sbp.tile([P, 256], f32, tag="dump", name=f"hf{b}")
                        nc.scalar.activation(hf[:], haloL[:, b, :], AF.Identity)
                        nc.sync.dma_start(dbg_d[:, b, 0:256], hf[:])
                        nc.sync.dma_start(dbg_d[:, b, 256:768],
                                          s2own[:, b, :].bitcast(f32))
                        hf2 = sbp.tile([P, 256], f32, tag="dump", name=f"hg{b}")
                        nc.scalar.activation(hf2[:], haloR[:, b, :], AF.Identity)
                        nc.sync.dma_start(dbg_d[:, b, 768:1024], hf2[:])
                    break
            else:
                with tc.tile_pool(name="outp", bufs=2) as op_, \
                     tc.tile_pool(name=f"psL2_{l}", bufs=1, space="PSUM") as psL:
                    dsts = [op_.tile([P, SL], f32, tag="xo", name=f"xo{b}")
                            for b in range(FB)]
                    _ln_T(nc, tc, None, sbp, psL, ones, negr, res2,
                          lambda b, _d=dsts: _d[b][:],
                          lambda b, _l=l: pp[_l][:, 6, b:b + 1],
                          lambda b, _l=l: pp[_l][:, 7, b:b + 1])
                    for b in range(FB):
                        nc.sync.dma_start(xout_d[b], dsts[b][:])

        if stage != 5:
            # touch every input + write xout so the NEFF keeps all I/O bound
            scr = constp.tile([P, 1024], f32, name="scratch")
            for l in range(L):
                for k in ("wq", "wk", "wv", "wo", "w1", "w2"):
                    nc.sync.dma_start(scr[:, 0:64],
                                      wd[l][k][0:P, 0:P].bitcast(f32))
            nc.sync.dma_start(xout_d[:],
                              slab_own[:, :, 0:SL].bitcast(f32)
                              .rearrange("p b t -> b p t"))

    nc.compile()
    _cache[stage] = nc
    return nc


# ---------------------------------------------------------------------------
def prep_inputs(inputs):
    ip = np.asarray(inputs["ip"]).astype(np.int64)
    mask = np.asarray(inputs["mask"]).astype(np.int32)
    we = np.asarray(inputs["word_emb"], dtype=np.float32)
    pe = np.asarray(inputs["pos_emb"], dtype=np.float32)
    te = np.asarray(inputs["type_emb"], dtype=np.float32)
    m = mask[0]
    pos_ids = (np.cumsum(m) * m + 1).astype(np.int64)

    def pack(v):  # [768] -> [128, 6]
        return np.ascontiguousarray(np.asarray(v, np.float32).reshape(FB, P).T)

    lnemb = np.ascontiguousarray(
        np.stack([pack(inputs["ln_emb_g"]), pack(inputs["ln_emb_b"])], axis=1))

    # shared per-layer tensors
    shared = {}
    for l in range(L):
        Wo = np.asarray(inputs["Wo"][l], np.float32)
        bv = np.asarray(inputs["bv"][l], np.float32)
        boeff = Wo.T @ bv + np.asarray(inputs["bo"][l], np.float32)
        ppk = np.ascontiguousarray(np.stack([
            pack(np.asarray(inputs["bq"][l], np.float32) * 0.125),
            pack(inputs["bk"][l]), pack(boeff), pack(inputs["b2"][l]),
            pack(inputs["ln1_g"][l]), pack(inputs["ln1_b"][l]),
            pack(inputs["ln2_g"][l]), pack(inputs["ln2_b"][l])], axis=1))
        for k, v in [("wq", inputs["Wq"][l]), ("wk", inputs["Wk"][l]),
                     ("wv", inputs["Wv"][l]), ("wo", Wo),
                     ("w1", inputs["W1"][l]), ("w2", inputs["W2"][l])]:
            shared[f"{k}{l}"] = np.ascontiguousarray(
                np.asarray(v, np.float32)).astype(ml_dtypes.bfloat16)
        shared[f"pp{l}"] = ppk
        shared[f"b1_{l}"] = np.ascontiguousarray(
            np.asarray(inputs["b1"][l], np.float32).reshape(FFB, P).T)
    shared["onesr"] = np.ones((P, P), np.float32)
    shared["negr"] = np.full((P, P), -1.0, np.float32)
    shared["onesb"] = np.ones((P, P), ml_dtypes.bfloat16)
    shared["lnemb"] = lnemb

    in_maps = []
    pr = np.arange(P)
    for c in range(NCORES):
        t0 = c * SL - W
        tt = np.clip(np.arange(t0, t0 + SLAB), 0, S - 1)
        xemb = we[ip[0, tt]] + pe[pos_ids[tt]] + te[0]

        bm = np.zeros((P, 2, FB, 256), np.float32)
        pj = pr[:, None, None]
        jj = np.arange(FB)[None, :, None]
        qq = np.arange(256)[None, None, :]
        rel = 128 * jj + pj - qq
        band = (rel >= 0) & (rel <= 2 * W)
        for ch in range(2):
            kg = c * SL + 256 * ch - W + 128 * jj + pj + 0 * qq
            valid = (kg >= 0) & (kg < S) & (m[np.clip(kg, 0, S - 1)] == 1)
            bm[:, ch][band & valid] = 1.0

        hidx = np.zeros((P, 2), np.int32)
        cL, cR = max(c - 1, 0), min(c + 1, NCORES - 1)
        hidx[:, 0] = (cL * 2 + 1) * P + pr
        hidx[:, 1] = (cR * 2 + 0) * P + pr

        im = dict(shared)
        im["xemb"] = np.ascontiguousarray(xemb, np.float32)
        im["bmask"] = bm.astype(ml_dtypes.bfloat16)
        im["hidx"] = hidx
        in_maps.append(im)
    return in_maps


def kernel(**inputs):
    stage = int(os.environ.get("KSTAGE", "5"))
    nc = build(stage)
    in_maps = prep_inputs(inputs)
    res = run_bass_kernel_spmd(nc, in_maps, list(range(NCORES)))
    outs = []
    for c in range(NCORES):
        xo = res.results[c]["xout"]  # [6, 128, 512]
        outs.append(np.transpose(xo, (2, 0, 1)).reshape(SL, HID))
    return np.concatenate(outs, axis=0)[None].astype(np.float32)
